# revision 22
# baseline (speedup 1.0000x reference)
"""EntityAttention Trainium2 kernel (nn_EntityAttention_31525059952740), v3.

Math (per (batch, entity) group n, all 64 events e):
  q = (events @ Wq.T + bq) * scale            shared across n     [64, 512]
  scoresT[s, (h,e)] = toks_b @ wtil           wtil = (q_h @ Wk_h) folded host-side
  E = exp(scoresT)                            shared by all 16 entities of the batch
  denom[ent, he] = masks.T @ E    (PE matmul), srec = 1/denom
  attnT[ent] = E * mask_ent  (DVE/Act/Pool, bf16)
  po[dh, (ent,e)] = vT @ attnT  (PE), outT = po * srec_bcast  (DVE)
  O[(ent,e), :] = outT.T @ WoT  (PE) -> bf16 -> DRAM

All matmuls run in bf16 (1 cycle/row on TRN2 PE regardless of free size;
f32 accumulate in PSUM). rel tolerance is 2e-2; bf16 keeps us ~3e-3.

Schedule notes (vs v2): PE is the bottleneck engine, so everything is
arranged to start PE early and keep it fed:
  - "head" DMA packs wtil + toksT chunk 0 so the first matmul's operands
    arrive in one transfer; remaining toks chunks land JIT in the order
    the scores loop consumes them (hc 0,2,3,1; c2 rides the gpsimd SWDGE
    queue which doesn't contend for the HWDGE).
  - warmup matmuls on scratch data ramp the PE clock out of its low
    p-state before real data arrives; a scratch activation preloads the
    Exp/Copy act table so the first real exp doesn't eat the 1.3us load.
  - srec store + broadcast loads ride SP right after the wv/wo issues;
    output DMAs are split 2-pairs-per-DMA across SP and Act so no single
    sequencer serializes the tail.
"""

import numpy as np
import ml_dtypes

import concourse.bass as bass
import concourse.tile as tile
import concourse.mybir as mybir
from concourse import bacc
from concourse.bass_utils import run_bass_kernel_spmd

NB, SL, NH, EN, NE, HEADS = 8, 512, 512, 16, 64, 2
DH = NH // HEADS          # 256
P = 128
NCHUNK = NH // P          # 4 chunks of the hidden dim
SCHUNK = SL // P          # 4 chunks of the sequence dim
SCALE = 1.0 / np.sqrt(DH).astype(np.float32)

F32 = mybir.dt.float32
BF16 = mybir.dt.bfloat16
BF = ml_dtypes.bfloat16

N_WARMUP = 16

_CACHE = {}


def _bcast_ap(ap_in, axis_len, pos):
    """Insert a stride-0 axis of length axis_len at free-dim position pos."""
    ap = list(ap_in.ap)
    ap.insert(pos, [0, axis_len])
    return bass.AP(tensor=ap_in.tensor, offset=ap_in.offset, ap=ap)


def _build():
    nc = bacc.Bacc("TRN2", target_bir_lowering=False, debug=False, num_devices=NB)

    HE = HEADS * NE  # 128
    # head = [wtil (4*128) | toksT chunk0, s-chunk0 (128)] per partition
    head_d = nc.dram_tensor("head", [P, NCHUNK * HE + P], BF16,
                            kind="ExternalInput").ap()
    c0rest_d = nc.dram_tensor("c0rest", [P, SL - P], BF16,
                              kind="ExternalInput").ap()
    toks1_d = nc.dram_tensor("toks1", [P, SL], BF16, kind="ExternalInput").ap()
    toks2_d = nc.dram_tensor("toks2", [P, SL], BF16, kind="ExternalInput").ap()
    toks3_d = nc.dram_tensor("toks3", [P, SL], BF16, kind="ExternalInput").ap()
    wv_d = nc.dram_tensor("wv", [P, NCHUNK, NH], BF16, kind="ExternalInput").ap()
    wo_d = nc.dram_tensor("wo", [P, NCHUNK, NH], BF16, kind="ExternalInput").ap()
    maskb_d = nc.dram_tensor("maskb", [P, SCHUNK, EN], BF16, kind="ExternalInput").ap()
    maskf_d = nc.dram_tensor("maskf", [P, SCHUNK, EN], F32, kind="ExternalInput").ap()
    out_d = nc.dram_tensor("out", [EN * NE, NH], BF16, kind="ExternalOutput").ap()

    EXP = mybir.ActivationFunctionType.Exp
    CPY = mybir.ActivationFunctionType.Copy

    with tile.TileContext(nc) as tc:
        with (
            nc.allow_low_precision(reason="bf16 pipeline; rel tolerance 2e-2"),
            tc.tile_pool(name="wpool", bufs=1) as wpool,
            tc.tile_pool(name="sb", bufs=1) as sb,
            tc.tile_pool(name="ostage", bufs=4) as ostage,
            tc.tile_pool(name="pacc", bufs=5, space="PSUM") as pacc,
            tc.tile_pool(name="pscore", bufs=2, space="PSUM") as pscore,
            tc.tile_pool(name="psS", bufs=1, space="PSUM") as psS,
            tc.tile_pool(name="dram", bufs=1, space="DRAM") as dram,
        ):
            # ---------- input DMAs ----------
            # SP: head -> toks3 -> wv(01) -> wv(23) -> wo -> srec store/loads -> outs
            # Act: toks1 -> maskb -> maskf (then compute)
            # Pool: toks2 via SWDGE (doesn't contend on HWDGE)
            head_t = wpool.tile([P, NCHUNK * HE + P], BF16, tag="head")
            nc.sync.dma_start(head_t[:], head_d)
            wtil_t = head_t[:, 0:NCHUNK * HE].rearrange("p (c e) -> p c e", c=NCHUNK)
            t3 = wpool.tile([P, SL], BF16, tag="toks3")
            nc.sync.dma_start(t3[:], toks3_d)
            c0rest = wpool.tile([P, SL - P], BF16, tag="c0rest")
            nc.sync.dma_start(c0rest[:], c0rest_d)

            t1 = wpool.tile([P, SL], BF16, tag="toks1")
            nc.scalar.dma_start(t1[:], toks1_d)
            t2 = wpool.tile([P, SL], BF16, tag="toks2")
            nc.gpsimd.dma_start(t2[:], toks2_d)
            toksT_t = [None, t1[:], t2[:], t3[:]]

            wv_t = wpool.tile([P, NCHUNK, NH], BF16, tag="wv")
            nc.scalar.dma_start(wv_t[:, 0:2, :], wv_d[:, 0:2, :])
            nc.sync.dma_start(wv_t[:, 2:4, :], wv_d[:, 2:4, :])
            wo_t = wpool.tile([P, NCHUNK, NH], BF16, tag="wo")
            nc.sync.dma_start(wo_t[:], wo_d)
            maskb = wpool.tile([P, SCHUNK, EN], BF16, tag="maskb")
            nc.gpsimd.dma_start(maskb[:], maskb_d)
            maskf = wpool.tile([P, SCHUNK, EN], F32, tag="maskf")
            nc.gpsimd.dma_start(maskf[:], maskf_d)

            def toks(hc, sc):
                if hc == 0:
                    if sc == 0:
                        return head_t[:, NCHUNK * HE:NCHUNK * HE + P]
                    return c0rest[:, (sc - 1) * P:sc * P]
                return toksT_t[hc][:, sc * P:(sc + 1) * P]

            # ---------- PE warmup + act-table preload ----------
            scratch = sb.tile([P, P], BF16, tag="scratch")
            nc.vector.memset(scratch[:], 0.0)
            ones = sb.tile([1, HE], BF16, tag="ones")
            nc.vector.memset(ones[:], 1.0)
            pwarm = pacc.tile([P, NH], F32, tag="big", name="pwarm")
            for i in range(N_WARMUP):
                nc.tensor.matmul(pwarm[:, 0:P], scratch[:], scratch[:],
                                 start=True, stop=True, skip_group_check=True)

            # ---------- scoresT + exp + denominators, per s-chunk ----------
            # per-sc psum/sbuf tiles keep dependencies fine-grained: exp(sc)
            # fires right after its 4 accumulation matmuls, the S matmul for
            # sc right after that exp.
            def filler(n):
                for _ in range(n):
                    nc.tensor.matmul(pwarm[:, 0:P], scratch[:], scratch[:],
                                     start=True, stop=True,
                                     skip_group_check=True)

            # Scores, bank-safe: bank A runs sc0 fully then sc1; bank B runs
            # sc2 then sc3 (accumulation groups in a bank never interleave).
            # Chunk order within each sc follows DMA arrival (head, toks1,
            # toks3, c2-pool, c0rest); filler warmups absorb the known
            # arrival waits so the in-order PE queue never idles long.
            psAB = [pscore.tile([P, 2, HE], F32, tag="pss", name=f"pss{i}")
                    for i in range(2)]

            def smm(sc, hc, start, stop):
                nc.tensor.matmul(psAB[sc // 2][:, sc % 2, :], toks(hc, sc),
                                 wtil_t[:, hc, :], start=start, stop=stop)

            smm(0, 0, True, False)      # head: toks c0 sc0
            filler(7)
            smm(0, 1, False, False)     # toks1
            smm(2, 1, True, False)
            filler(2)
            smm(0, 3, False, False)     # toks3
            smm(2, 3, False, False)
            filler(2)
            smm(0, 2, False, True)      # c2 -> sc0 closed
            smm(2, 2, False, False)
            smm(1, 1, True, False)      # sc1 start (bank A free now)
            smm(1, 3, False, False)
            smm(1, 2, False, False)
            smm(2, 0, False, True)      # c0rest -> sc2 closed
            smm(1, 0, False, True)      # sc1 closed
            smm(3, 1, True, False)      # sc3 (bank B free)
            smm(3, 3, False, False)
            smm(3, 2, False, False)
            smm(3, 0, False, True)
            e_sbs = []
            for sc in [0, 2, 1, 3]:
                e = sb.tile([P, HE], BF16, tag=f"E{sc}", name=f"E{sc}")
                nc.scalar.activation(e[:], psAB[sc // 2][:, sc % 2, :], EXP)
                e_sbs.append((sc, e))
            e_sbs = [e for _, e in sorted(e_sbs)]
            pS = psS.tile([EN, HE], F32, tag="pS")
            for i, sc in enumerate([0, 2, 1, 3]):
                nc.tensor.matmul(pS[:], maskb[:, sc, :], e_sbs[sc][:],
                                 start=(i == 0), stop=(i == 3))
            srec = sb.tile([EN, HE], BF16, tag="srec")
            nc.vector.reciprocal(srec[:], pS[:])
            # flatten srec onto partition 0 (sbuf->sbuf DMA) so the K=1
            # broadcast matmuls below read from a legal base partition
            srecF = sb.tile([1, EN * HE], BF16, tag="srecF")
            nc.sync.dma_start(srecF[:], srec[:])

            # ---------- V = toks @ WvT ----------
            vcopy_eng = [nc.scalar, nc.vector, nc.scalar, nc.scalar]
            vs = []
            for sc in range(SCHUNK):
                pv = pacc.tile([P, NH], F32, tag="big", name=f"pv{sc}")
                for hc in range(NCHUNK):
                    nc.tensor.matmul(pv[:], toks(hc, sc), wv_t[:, hc, :],
                                     start=(hc == 0), stop=(hc == NCHUNK - 1))
                v = sb.tile([P, NH], BF16, tag=f"v{sc}", name=f"v{sc}")
                if vcopy_eng[sc] is nc.scalar:
                    nc.scalar.activation(v[:], pv[:], CPY)
                else:
                    vcopy_eng[sc].tensor_copy(v[:], pv[:])
                vs.append(v)

            # ---------- srec broadcast across partitions via K=1 matmuls ----
            # srbc[p, ent, e] = srec[ent, h*64+e] for all p; ones[1,HE] outer
            # product, one matmul per entity. Replaces the DRAM roundtrip.
            srbc_sb = {}
            srbc_copy_eng = {(0, 0): nc.scalar, (0, 1): nc.vector,
                             (1, 0): nc.scalar, (1, 1): nc.vector}
            for g8 in range(2):
                for h in range(HEADS):
                    pb = pacc.tile([P, 8, NE], F32, tag="big",
                                   name=f"srbcp{g8}_{h}")
                    for k in range(8):
                        ent = g8 * 8 + k
                        off = ent * HE + h * NE
                        nc.tensor.matmul(pb[:, k, :], ones[:],
                                         srecF[0:1, off:off + NE],
                                         start=True, stop=True)
                    t = sb.tile([P, 8, NE], BF16, tag=f"srbc{g8}_{h}",
                                name=f"srbc{g8}_{h}")
                    eng = srbc_copy_eng[(g8, h)]
                    if eng is nc.scalar:
                        nc.scalar.activation(t[:], pb[:], CPY)
                    else:
                        eng.tensor_copy(t[:], pb[:])
                    srbc_sb[(g8, h)] = t

            # ---------- masked attnT (bf16), per (g8, sc) ----------
            # DVE: batch-8 tensor_tensor; Pool: batch-4 (SBUF-only engine);
            # Act: per-entity activation-with-scale.
            mask_plan = {(0, 0): "dve8", (0, 1): "pool4", (0, 2): "dve8",
                         (0, 3): "act", (1, 0): "dve8", (1, 1): "pool4",
                         (1, 2): "dve8", (1, 3): "act"}
            attnT = {}
            for g8 in range(2):
                for sc in range(SCHUNK):
                    a = sb.tile([P, 8, HE], BF16, tag=f"attnT{g8}_{sc}",
                                name=f"attnT{g8}_{sc}")
                    e_slice = e_sbs[sc][:]
                    kind = mask_plan[(g8, sc)]
                    if kind == "act":
                        for k in range(8):
                            ent = g8 * 8 + k
                            nc.scalar.activation(a[:, k, :], e_slice, CPY,
                                                 scale=maskf[:, sc, ent:ent + 1])
                    elif kind == "dve8":
                        e_bc = _bcast_ap(e_slice, 8, 1)
                        m_slice = maskb[:, sc, g8 * 8:(g8 + 1) * 8]
                        m_bc = _bcast_ap(m_slice, HE, 2)
                        nc.vector.tensor_mul(a[:], e_bc, m_bc)
                    else:
                        for half in range(2):
                            e_bc = _bcast_ap(e_slice, 4, 1)
                            m_slice = maskb[:, sc,
                                            g8 * 8 + half * 4:g8 * 8 + half * 4 + 4]
                            m_bc = _bcast_ap(m_slice, HE, 2)
                            nc.gpsimd.tensor_mul(a[:, half * 4:half * 4 + 4, :],
                                                 e_bc, m_bc)
                    attnT[(g8, sc)] = a

            # ---------- PV -> normalize (DVE) ----------
            outT = []
            for g8 in range(2):
                o = sb.tile([P, NCHUNK, 8, NE], BF16, tag=f"outT{g8}",
                            name=f"outT{g8}")
                outT.append(o)
            for g8 in range(2):
                for h in range(HEADS):
                    for dc in range(2):
                        hcd = 2 * h + dc
                        po = pacc.tile([P, 8 * NE], F32, tag="big",
                                       name=f"po{g8}_{hcd}")
                        for sc in range(SCHUNK):
                            nc.tensor.matmul(
                                po[:],
                                vs[sc][:, hcd * P:(hcd + 1) * P],
                                attnT[(g8, sc)][:, :, h * NE:(h + 1) * NE],
                                start=(sc == 0), stop=(sc == SCHUNK - 1))
                        nc.vector.tensor_mul(
                            outT[g8][:, hcd, :, :],
                            po[:].rearrange("p (a b) -> p a b", a=8),
                            srbc_sb[(g8, h)][:])

            # ---------- O = outT.T @ WoT -> bf16 -> DRAM ----------
            ocopy_eng = [nc.scalar, nc.vector, nc.scalar, nc.vector,
                         nc.scalar, nc.vector, nc.scalar, nc.vector]
            odma_eng = [nc.sync, nc.scalar, nc.sync, nc.scalar,
                        nc.sync, nc.scalar, nc.sync, nc.scalar]
            for g8 in range(2):
                for lp in range(4):
                    pair = g8 * 4 + lp
                    pO = pacc.tile([P, NH], F32, tag="big", name=f"pO{pair}")
                    for hc in range(NCHUNK):
                        nc.tensor.matmul(pO[:], outT[g8][:, hc, 2 * lp:2 * lp + 2, :],
                                         wo_t[:, hc, :],
                                         start=(hc == 0), stop=(hc == NCHUNK - 1))
                    ob = ostage.tile([P, NH], BF16, tag="ob", name=f"ob{pair}")
                    if ocopy_eng[pair] is nc.scalar:
                        nc.scalar.activation(ob[:], pO[:], CPY)
                    else:
                        ocopy_eng[pair].tensor_copy(ob[:], pO[:])
                    odma_eng[pair].dma_start(
                        out_d[pair * P:(pair + 1) * P, :], ob[:])

    nc.compile()
    return nc


def _get_nc():
    if "nc" not in _CACHE:
        _CACHE["nc"] = _build()
    return _CACHE["nc"]


def _fast_run(nc, in_maps):
    """Repeat-call path: same PJRT execution as run_bass_kernel_spmd/
    bass2jax.run_bass_via_pjrt, but with the jitted shard_map cached so
    repeat kernel() calls skip retracing/relowering."""
    import jax
    from jax.sharding import Mesh, PartitionSpec
    from jax.experimental.shard_map import shard_map
    import concourse.mybir as mybir_
    from concourse import bass2jax

    if "runner" not in _CACHE:
        bass2jax.install_neuronx_cc_hook()
        part_name = (nc.partition_id_tensor.name
                     if nc.partition_id_tensor else None)
        in_names, out_names, out_avals = [], [], []
        for alloc in nc.m.functions[0].allocations:
            if not isinstance(alloc, mybir_.MemoryLocationSet):
                continue
            name = alloc.memorylocations[0].name
            if alloc.kind == "ExternalInput":
                if name != part_name:
                    in_names.append(name)
            elif alloc.kind == "ExternalOutput":
                out_names.append(name)
                out_avals.append(jax.core.ShapedArray(
                    tuple(alloc.tensor_shape), mybir_.dt.np(alloc.dtype)))
        n_params = len(in_names)
        all_in_names = in_names + out_names
        if part_name is not None:
            all_in_names = all_in_names + [part_name]

        def _body(*args):
            operands = list(args)
            if part_name is not None:
                operands.append(bass2jax.partition_id_tensor())
            outs = bass2jax._bass_exec_p.bind(
                *operands,
                out_avals=tuple(out_avals),
                in_names=tuple(all_in_names),
                out_names=tuple(out_names),
                lowering_input_output_aliases=(),
                sim_require_finite=True,
                sim_require_nnan=True,
                nc=nc,
            )
            return tuple(outs)

        devices = jax.devices()[:NB]
        mesh = Mesh(np.asarray(devices), ("core",))
        n_outs = len(out_names)
        sharded = jax.jit(
            shard_map(_body, mesh=mesh,
                      in_specs=(PartitionSpec("core"),) * (n_params + n_outs),
                      out_specs=(PartitionSpec("core"),) * n_outs,
                      check_rep=False),
            donate_argnums=tuple(range(n_params, n_params + n_outs)),
            keep_unused=True,
        )
        _CACHE["runner"] = (sharded, in_names, out_names, out_avals)

    sharded, in_names, out_names, out_avals = _CACHE["runner"]
    concat_in = [
        np.concatenate([np.asarray(m[name]) for m in in_maps], axis=0)
        for name in in_names
    ]
    concat_zeros = [
        np.zeros((NB * av.shape[0], *av.shape[1:]), av.dtype)
        for av in out_avals
    ]
    out_arrs = sharded(*concat_in, *concat_zeros)
    return [
        {name: np.asarray(out_arrs[i]).reshape(NB, *out_avals[i].shape)[c]
         for i, name in enumerate(out_names)}
        for c in range(NB)
    ]


def kernel(tokens_embed, entities, events_embed, entity_num, entity_masks,
           select_event, Wq, Wk, Wv, bq, bk, bv, Wo, bo):
    tokens_embed = np.asarray(tokens_embed, dtype=np.float32)
    entities = np.asarray(entities)
    events_embed = np.asarray(events_embed, dtype=np.float32)
    entity_masks = np.asarray(entity_masks)
    select_event = np.asarray(select_event)
    Wq = np.asarray(Wq, dtype=np.float32)
    Wk = np.asarray(Wk, dtype=np.float32)
    Wv = np.asarray(Wv, dtype=np.float32)
    Wo = np.asarray(Wo, dtype=np.float32)
    bq = np.asarray(bq, dtype=np.float32)
    bk = np.asarray(bk, dtype=np.float32)
    bv = np.asarray(bv, dtype=np.float32)
    bo = np.asarray(bo, dtype=np.float32)

    nc = _get_nc()

    q_s = (events_embed @ Wq.T + bq) * SCALE          # [NE, NH]
    # fold the K projection into the query side (bk cancels in softmax):
    # wtil[hid, (h,e)] = sum_dout_in_head Wk[dout, hid] * q_s[e, dout]
    wtil = np.empty((NH, HEADS * NE), dtype=np.float32)
    for h in range(HEADS):
        hs = slice(h * DH, (h + 1) * DH)
        wtil[:, h * NE:(h + 1) * NE] = (q_s[:, hs] @ Wk[hs, :]).T
    wtil_pc = np.ascontiguousarray(
        wtil.reshape(NCHUNK, P, HEADS * NE).transpose(1, 0, 2)).astype(BF)
    wtil_flat = wtil_pc.reshape(P, NCHUNK * HEADS * NE)
    # attn rows sum to 1, so the bv term of out contributes bv @ Wo.T to O;
    # the whole output bias is applied host-side after the gather.
    bo2 = (bo + bv @ Wo.T).astype(np.float32)
    shared = {
        "wv": np.ascontiguousarray(
            Wv.T.reshape(NCHUNK, P, NH).transpose(1, 0, 2)).astype(BF),
        "wo": np.ascontiguousarray(
            Wo.T.reshape(NCHUNK, P, NH).transpose(1, 0, 2)).astype(BF),
    }
    in_maps = []
    for c in range(NB):
        # maskT[p, sc, ent] = entities[c, ent, sc*128 + p]
        m = entities[c].astype(np.float32)            # [EN, SL]
        mT = np.ascontiguousarray(
            m.reshape(EN, SCHUNK, P).transpose(2, 1, 0))
        toks_pc = np.ascontiguousarray(
            tokens_embed[c].T.reshape(NCHUNK, P, SL).transpose(1, 0, 2)).astype(BF)
        head = np.concatenate([wtil_flat, toks_pc[:, 0, 0:P]], axis=1)
        in_maps.append({
            "head": np.ascontiguousarray(head),
            "c0rest": np.ascontiguousarray(toks_pc[:, 0, P:]),
            "toks1": np.ascontiguousarray(toks_pc[:, 1, :]),
            "toks2": np.ascontiguousarray(toks_pc[:, 2, :]),
            "toks3": np.ascontiguousarray(toks_pc[:, 3, :]),
            "maskb": mT.astype(BF),
            "maskf": mT,
            **shared,
        })

    if "ran_once" not in _CACHE:
        res = run_bass_kernel_spmd(nc, in_maps, core_ids=list(range(NB)))
        results = res.results
        _CACHE["ran_once"] = True
    else:
        results = _fast_run(nc, in_maps)
    full = np.concatenate([results[c]["out"] for c in range(NB)],
                          axis=0).astype(np.float32)
    full += bo2[None, :]
    # full[(b*EN + ent)*NE + e] = attention output for group (b, ent), event e

    # ragged selection (mirrors the reference indexing; identity for the
    # all-ones masks produced by setup_inputs)
    assert int(entity_num) == EN
    entity_index = np.flatnonzero(entity_masks.reshape(-1))
    pair_sel = (select_event[:, None, :] & entity_masks[:, :, None])
    pair_sel = pair_sel.reshape(-1, NE)[entity_index].reshape(-1)
    event_entity_index = np.flatnonzero(pair_sel)

    sel_rows = (entity_index[:, None] * NE + np.arange(NE)[None, :]).reshape(-1)
    return full[sel_rows][event_entity_index]
